# revision 40
# baseline (speedup 1.0000x reference)
"""Two-layer GCN (PyG GCNConv semantics) on 8 Trainium2 NeuronCores.

Strategy: nodes are sharded row-wise across the 8 cores; edges are
partitioned by destination node so the segment-sum stays local.

Layer 1 gathers raw x rows (128 f16 = 256B, the SWDGE minimum, fully
used) directly from a replicated DRAM table and folds W1 in after
aggregation: PT[in, dst] = sum_e x[src_e] * S, then acc1 = relu(PT.T @ W1
+ b1).  This removes both the first pre-matmul and the first (25.6MB)
AllGather from the critical path; layer-1 gathers start at t=0.

Layer 2 all-gathers the compact acc1 [nloc, 64] f16 (12.8MB full) viewed
as a [50000, 128] pair table (256B rows hold two nodes).  Edges are
sorted by (dst window, pair range, src parity) so each 128-edge tile
statically consumes one 64-wide half of its gathered rows; W2 is folded
in per window after aggregation, then bias + log_softmax.

Gather pipeline: batched SWDGE dma_gather (int16 indices, CHUNK*128 rows
per instruction), one queue per index range.  Aggregation per 128-node
destination window: a norm-weighted one-hot S[e, j] = norm[e] *
(dst_rel[e] == j) built in one DVE tensor_scalar per tile (1 in 6 on the
scalar engine), PE matmuls accumulate into PSUM.

Edge bookkeeping (sorting, slot assignment, padding so all 8 cores share
one instruction stream) is host-side numpy index work; all floating-point
math on features runs on device.
"""

import math

import numpy as np

import concourse.bass as bass
import concourse.mybir as mybir
import concourse.tile as tile
from concourse import library_config
from concourse.bass_utils import run_bass_kernel_spmd

N_NODES = 100000
N_EDGES = 1600000
IN_DIM, HID_DIM, OUT_DIM = 128, 64, 40
N_CORES = 8

RNG = 25000          # table rows per gather range (int16 index limit)
CHUNK = 32           # tiles per dma_gather instruction
GBUFS = 8            # gather chunk buffers in flight
SBUFS = 16
ACT_NTH = 0          # if >0, every ACT_NTH-th S-build goes to scalar engine
PL_NTH = 0           # if >0, every PL_NTH-th (offset 1) S-build on gpsimd
SB_IOTA = False      # S-build reads iota from SBUF f16 (fast DVE mode) vs PSUM
AGC = 2              # AllGather chunks (table row order is chunk-major)

F32 = mybir.dt.float32
F16 = mybir.dt.float16
I16 = mybir.dt.int16


def _split_long_waits(nc, max_waits=1):
    """This toolchain's codegen rejects instructions carrying more than one
    semaphore wait; move extra waits onto preceding same-engine no-ops."""
    cnt = 0
    for bb in nc.main_func.blocks:
        i = 0
        insts = bb.instructions
        while i < len(insts):
            ins = insts[i]
            si = ins.sync_info
            if si is not None and si.on_wait and len(si.on_wait) > max_waits:
                waits = list(si.on_wait)
                keep = waits[-max_waits:]
                extra = waits[:-max_waits]
                si.on_wait = keep
                new_insts = []
                for j in range(0, len(extra), max_waits):
                    chunk = extra[j : j + max_waits]
                    nop = mybir.InstNoOp(
                        name=f"{ins.name}-waitsplit-{j}",
                        engine=ins.engine,
                        ins=[],
                        outs=[],
                        sync_info=mybir.SyncInfo(on_wait=chunk, on_update=[]),
                    )
                    new_insts.append(nop)
                insts[i:i] = new_insts
                i += len(new_insts)
                cnt += len(new_insts)
            i += 1
    return cnt


def _mkstream(n_cores, wn, core, w, gsub, n_gsub, queue_of_gsub, nq,
              idxval, drel, nrm):
    """Group edges by (core, dst window, subgroup); lay out 128-slot tiles
    shared across cores (single SPMD program).  Tile ids are queue-major;
    within a queue they follow consumption order (w, then subgroup, then
    k), so per-queue gathers are contiguous slices.

    Returns device-layout index/scalar arrays plus the tile layout.
    """
    key = (core.astype(np.int64) * wn + w) * n_gsub + gsub
    order = np.argsort(key, kind="stable")
    counts = np.bincount(key, minlength=n_cores * wn * n_gsub).reshape(
        n_cores, wn, n_gsub
    )
    k_wg = (counts.max(axis=0) + 127) // 128  # [wn, n_gsub]

    gs_of_q = [[g for g in range(n_gsub) if queue_of_gsub[g] == q]
               for q in range(nq)]
    tile_first = np.zeros((wn, n_gsub), dtype=np.int64)
    t = 0
    t_start = []
    for q in range(nq):
        t_start.append(t)
        for ww in range(wn):
            for g in gs_of_q[q]:
                tile_first[ww, g] = t
                t += int(k_wg[ww, g])
    T = t
    t_start.append(T)

    grp_start = np.zeros(n_cores * wn * n_gsub, dtype=np.int64)
    grp_start[1:] = np.cumsum(counts.reshape(-1))[:-1]
    rank = np.arange(len(order), dtype=np.int64) - grp_start[key[order]]

    w_o = w[order]
    g_o = gsub[order]
    slot = (tile_first[w_o, g_o] + (rank >> 7)) * 128 + (rank & 127)

    n_slots = T * 128
    idx_a = np.zeros((n_cores, n_slots), dtype=np.int16)
    drel_a = np.zeros((n_cores, n_slots), dtype=np.float32)
    nrm_a = np.zeros((n_cores, n_slots), dtype=np.float32)
    c_o = core[order]
    idx_a[c_o, slot] = idxval[order].astype(np.int16)
    drel_a[c_o, slot] = drel[order]
    nrm_a[c_o, slot] = nrm[order]

    # device layouts:
    #   idx: [128, T*8] int16, slot s at [16g + s%16, s//16], g=0..7
    #   drel/nrm: [128, T] f32, slot s at [s%128, s//128]
    idx_dev, drel_dev, nrm_dev = [], [], []
    for c in range(n_cores):
        base = idx_a[c].reshape(n_slots // 16, 16).T
        idx_dev.append(np.tile(base, (8, 1)).copy())
        drel_dev.append(drel_a[c].reshape(T, 128).T.copy())
        nrm_dev.append(nrm_a[c].reshape(T, 128).T.copy())

    return {
        "T": T,
        "t_start": t_start,
        "nq": nq,
        "k_wg": k_wg,
        "tile_first": tile_first,
        "gs_of_q": gs_of_q,
        "queue_of_gsub": list(queue_of_gsub),
        "idx": idx_dev,
        "drel": drel_dev,
        "nrm": nrm_dev,
        # host-side slot arrays for preprocessing verification
        "host_slot": (c_o, slot, order),
    }


def _preprocess(edge_index, n_nodes, n_cores):
    nloc = n_nodes // n_cores
    wn = math.ceil(nloc / 128)

    src = np.asarray(edge_index[0], dtype=np.int64)
    dst = np.asarray(edge_index[1], dtype=np.int64)
    loop = np.arange(n_nodes, dtype=np.int64)
    src_all = np.concatenate([src, loop])
    dst_all = np.concatenate([dst, loop])

    deg = np.bincount(dst_all, minlength=n_nodes).astype(np.float64)
    dis = np.where(deg > 0, 1.0 / np.sqrt(deg), 0.0)
    norm = (dis[src_all] * dis[dst_all]).astype(np.float32)

    core = dst_all // nloc
    dloc = dst_all - core * nloc
    w = dloc >> 7
    drel = (dloc & 127).astype(np.float32)

    # layer 1: gather x rows; ranges of RNG node rows.  Self-loops are
    # excluded — they are fed from the resident local x window on device
    # (their diagonal S carries dinv = 1/deg).
    ne = len(src)
    r1 = src // RNG
    nr1 = int(src_all.max()) // RNG + 1
    s1 = _mkstream(
        n_cores, wn, core[:ne], w[:ne],
        gsub=r1, n_gsub=nr1, queue_of_gsub=list(range(nr1)), nq=nr1,
        idxval=src - r1 * RNG, drel=drel[:ne], nrm=norm[:ne],
    )
    s1["par"] = None
    s1["rng"] = RNG

    # per-window dinv columns for the layer-1 self-loop diagonal tiles
    dinv = (dis * dis).astype(np.float32)  # 1/deg
    npad = wn * 128
    dinvw = []
    for c in range(n_cores):
        dpad = np.zeros(npad, dtype=np.float32)
        dpad[:nloc] = dinv[c * nloc : (c + 1) * nloc]
        dinvw.append(dpad.reshape(wn, 128).T.copy())  # [128, wn]

    # layer 2: gather acc1 pair rows (two 64-wide nodes per 256B row).
    # Table row order is AllGather-chunk-major: chunk k holds pair-rows
    # [k*PHALF, (k+1)*PHALF) of every core, so each of the AGC collectives
    # reads/writes a contiguous block and chunk == gather range.
    # subgroup = (chunk, parity) so tiles are single-half.
    phalf = nloc // 2 // AGC
    sc = src_all // nloc          # core of source node
    sll = (src_all % nloc) >> 1   # local pair row
    k2 = sll // phalf             # AG chunk
    row2 = k2 * (n_cores * phalf) + sc * phalf + (sll - k2 * phalf)
    assert n_cores * phalf <= RNG
    par = (src_all & 1).astype(np.int64)
    g2 = k2 * 2 + par
    s2 = _mkstream(
        n_cores, wn, core, w,
        gsub=g2, n_gsub=AGC * 2,
        queue_of_gsub=[g // 2 for g in range(AGC * 2)], nq=AGC,
        idxval=row2 - k2 * (n_cores * phalf), drel=drel, nrm=norm,
    )
    s2["par"] = [g % 2 for g in range(AGC * 2)]  # parity per subgroup
    s2["rng"] = n_cores * phalf   # table rows per range (= per AG chunk)

    return {
        "nloc": nloc,
        "wn": wn,
        "s1": s1,
        "s2": s2,
        "dinvw": dinvw,
        "src_all": src_all,
        "dst_all": dst_all,
        "norm_all": norm,
    }


def _build_nc(meta, n_nodes, hid, out_dim, in_dim, n_cores, rounds=1):
    nloc = meta["nloc"]
    wn = meta["wn"]
    s1, s2 = meta["s1"], meta["s2"]
    npair = n_nodes // 2

    nc = bass.Bass(num_devices=n_cores, num_swdge_queues=4)

    xtab = nc.dram_tensor("xtab", [n_nodes, in_dim], F16, kind="ExternalInput")
    xloc = nc.dram_tensor("xloc", [128, wn * 128], F16, kind="ExternalInput")
    dinvw = nc.dram_tensor("dinvw", [128, wn], F32, kind="ExternalInput")
    iotap_in = nc.dram_tensor("iotap", [128, 1], F32, kind="ExternalInput")
    idx1 = nc.dram_tensor("idx1", [128, s1["T"] * 8], I16, kind="ExternalInput")
    drel1 = nc.dram_tensor("drel1", [128, s1["T"]], F32, kind="ExternalInput")
    nrm1 = nc.dram_tensor("nrm1", [128, s1["T"]], F32, kind="ExternalInput")
    idx2 = nc.dram_tensor("idx2", [128, s2["T"] * 8], I16, kind="ExternalInput")
    drel2 = nc.dram_tensor("drel2", [128, s2["T"]], F32, kind="ExternalInput")
    nrm2 = nc.dram_tensor("nrm2", [128, s2["T"]], F32, kind="ExternalInput")
    w1 = nc.dram_tensor("w1", [in_dim, hid], F16, kind="ExternalInput")
    w2 = nc.dram_tensor("w2", [hid, out_dim], F16, kind="ExternalInput")
    b1rep = nc.dram_tensor("b1rep", [128, hid], F32, kind="ExternalInput")
    b2rep = nc.dram_tensor("b2rep", [128, out_dim], F32, kind="ExternalInput")
    iota_in = nc.dram_tensor("iota", [128, 128], F16, kind="ExternalInput")
    out = nc.dram_tensor("out", [nloc, out_dim], F32, kind="ExternalOutput")

    eq = mybir.AluOpType.is_equal
    mul = mybir.AluOpType.mult

    with tile.TileContext(nc) as tc:
        with (
            tc.tile_pool(name="const", bufs=1) as cp,
            tc.tile_pool(name="gpool", bufs=GBUFS) as gp,
            tc.tile_pool(name="spool", bufs=SBUFS) as sp,
            tc.tile_pool(name="evac", bufs=6) as ep,
            tc.tile_pool(name="ps_agg", bufs=3, space="PSUM") as pa,
            tc.tile_pool(name="ps_mm", bufs=2, space="PSUM") as pm,
            tc.tile_pool(name="ps_const", bufs=1, space="PSUM") as pc,
            tc.tile_pool(name="dram", bufs=1, space="DRAM") as dp,
        ):
            nc.gpsimd.load_library(library_config.mlp)

            # ---- resident tensors ----
            def load(name, dram, shape, dt):
                t = cp.tile(shape, dt, name=name)
                nc.sync.dma_start(out=t[:], in_=dram[:])
                return t

            idx1_t = load("idx1t", idx1, [128, s1["T"] * 8], I16)
            drel1_t = load("drel1t", drel1, [128, s1["T"]], F32)
            nrm1_t = load("nrm1t", nrm1, [128, s1["T"]], F32)
            idx2_t = load("idx2t", idx2, [128, s2["T"] * 8], I16)
            drel2_t = load("drel2t", drel2, [128, s2["T"]], F32)
            nrm2_t = load("nrm2t", nrm2, [128, s2["T"]], F32)
            w1_t = load("w1t", w1, [in_dim, hid], F16)
            w2_t = load("w2t", w2, [hid, out_dim], F16)
            b1_t = load("b1t", b1rep, [128, hid], F32)
            b2_t = load("b2t", b2rep, [128, out_dim], F32)
            iota_t = load("iotat", iota_in, [128, 128], F16)
            xloc_t = load("xloct", xloc, [128, wn * 128], F16)
            dinvw_t = load("dinvwt", dinvw, [128, wn], F32)
            iotap_t = load("iotapt", iotap_in, [128, 1], F32)

            if ACT_NTH:
                # negated norms for the ACT-path S-build (scale = -nrm)
                nneg1_t = cp.tile([128, s1["T"]], F32, name="nneg1t")
                nc.vector.tensor_scalar_mul(
                    out=nneg1_t[:], in0=nrm1_t[:], scalar1=-1.0
                )
                nneg2_t = cp.tile([128, s2["T"]], F32, name="nneg2t")
                nc.vector.tensor_scalar_mul(
                    out=nneg2_t[:], in0=nrm2_t[:], scalar1=-1.0
                )
            else:
                nneg1_t = nneg2_t = None

            if SB_IOTA:
                # all-SBUF f16 operands let the DVE S-build use its fast
                # perf mode (risk: 2-port mode contends with GPSIMD SBUF
                # access for SWDGE descriptor rings)
                iota_s = iota_t
            else:
                # iota in PSUM keeps the DVE S-builds out of 2-port perf mode
                iota_ps = pc.tile([128, 128], F32)
                nc.scalar.activation(
                    out=iota_ps[:], in_=iota_t[:],
                    func=mybir.ActivationFunctionType.Identity,
                )
                iota_s = iota_ps

            acc1loc = dp.tile([nloc, hid], F16, name="acc1loc")

            nidx_regs = {}

            def nidx_reg(n):
                if n not in nidx_regs:
                    nidx_regs[n] = nc.gpsimd.to_reg(n)
                return nidx_regs[n]

            # SWDGE sem lanes (8) are assigned round-robin per Pool-DMA
            # instruction and each lane is locked to one hardware queue, so
            # rotate queues with the emission counter to keep lane->queue
            # mapping consistent.
            gq_counter = [0]

            def next_queue():
                q = gq_counter[0] % 4
                gq_counter[0] += 1
                return q

            def build_s(st, t, drel_t, nrm_t, nneg_t):
                """S[e, j] = norm[e] * (dst_rel[e] == j)."""
                s = sp.tile([128, 128], F16, tag="s", name="s")
                if ACT_NTH and t % ACT_NTH == ACT_NTH - 1:
                    # ACT path: relu(norm - norm*(drel-iota)^2) == norm iff eq
                    tmp = sp.tile([128, 128], F16, tag="stmp", name="stmp")
                    nc.scalar.activation(
                        out=tmp[:], in_=iota_t[:],
                        func=mybir.ActivationFunctionType.Square,
                        bias=drel_t[:, t : t + 1], scale=-1.0,
                    )
                    nc.scalar.activation(
                        out=s[:], in_=tmp[:],
                        func=mybir.ActivationFunctionType.Relu,
                        bias=nrm_t[:, t : t + 1], scale=nneg_t[:, t : t + 1],
                    )
                else:
                    eng = nc.vector
                    if PL_NTH and t % PL_NTH == 1:
                        eng = nc.gpsimd
                    eng.tensor_scalar(
                        out=s[:], in0=iota_s[:],
                        scalar1=drel_t[:, t : t + 1],
                        scalar2=nrm_t[:, t : t + 1],
                        op0=eq, op1=mul,
                    )
                return s

            def agg_pass(st, tables, table_rows, idx_t, drel_t, nrm_t, nneg_t,
                         layer, post_window=None, pre_hook=None):
                """Chunk-pipelined gather + per-window aggregation."""
                nq = st["nq"]
                t_start = st["t_start"]
                k_wg = st["k_wg"]
                tile_first = st["tile_first"]
                par_of = st["par"]
                issued = [0] * nq
                chunks = [[] for _ in range(nq)]

                rng = st["rng"]

                def ensure(q, tiles_needed):
                    T_q = t_start[q + 1] - t_start[q]
                    while issued[q] < tiles_needed:
                        t0 = t_start[q] + issued[q]
                        span = min(CHUNK, T_q - issued[q])
                        g = gp.tile([128, CHUNK, 128], F16, tag="g", name="g")
                        if len(tables) == 1:  # one table, per-range offsets
                            rows0 = q * rng
                            rows1 = min(rows0 + rng, table_rows)
                            tab = tables[0][rows0:rows1, :]
                        else:  # one table per range
                            tab = tables[q][:, :]
                        nc.gpsimd.dma_gather(
                            g[:, 0:span, :],
                            tab,
                            idx_t[:, t0 * 8 : (t0 + span) * 8],
                            span * 128,
                            nidx_reg(span * 128),
                            128,
                            single_packet=False,
                            queue_num=next_queue(),
                        )
                        chunks[q].append((g, t0, span))
                        issued[q] += span

                def gview(q, t):
                    while True:
                        g, t0, span = chunks[q][0]
                        if t < t0 + span:
                            return g[:, t - t0, :]
                        chunks[q].pop(0)

                if pre_hook is not None:
                    pre_hook(ensure)

                for w in range(wn):
                    # consumption order: subgroups by (queue, in-queue order)
                    todo = []  # (t, q, par)
                    ktot = 1 if layer == 1 else 0  # self-tile counts in L1
                    for q in range(nq):
                        for g in st["gs_of_q"][q]:
                            kw = int(k_wg[w][g])
                            if kw == 0:
                                continue
                            ensure(q, tile_first[w][g] - t_start[q] + kw)
                            p = 0 if par_of is None else par_of[g]
                            for k in range(kw):
                                todo.append((int(tile_first[w][g]) + k, q, p))
                            ktot += kw
                    if layer == 1:
                        pt = pa.tile([in_dim, 128], F32, tag="pagg")
                        # self-loop diagonal tile from the resident local x
                        # window: S_self = (iota == p) * dinv
                        s_self = sp.tile([128, 128], F16, tag="s", name="s")
                        nc.vector.tensor_scalar(
                            out=s_self[:], in0=iota_s[:],
                            scalar1=iotap_t[:],
                            scalar2=dinvw_t[:, w : w + 1],
                            op0=eq, op1=mul,
                        )
                        nc.tensor.matmul(
                            out=pt[:],
                            lhsT=xloc_t[:, w * 128 : (w + 1) * 128],
                            rhs=s_self[:],
                            start=True, stop=(ktot == 1),
                        )
                        ki0 = 1
                    else:
                        pt = pa.tile([hid, 128], F32, tag="pagg")
                        ki0 = 0
                    for ki, (t, q, p) in enumerate(todo, start=ki0):
                        gfull = gview(q, t)
                        if layer == 1:
                            gv = gfull[:, 0:in_dim]
                        else:
                            gv = gfull[:, p * hid : (p + 1) * hid]
                        s = build_s(st, t, drel_t, nrm_t, nneg_t)
                        nc.tensor.matmul(
                            out=pt[:], lhsT=gv, rhs=s[:],
                            start=(ki == 0), stop=(ki == ktot - 1),
                        )
                    rows = min(128, nloc - w * 128)
                    if layer == 1:
                        # acc1[w] = relu(PT.T @ W1 + b1)
                        pt_sb = ep.tile([in_dim, 128], F16, tag="ptsb")
                        nc.vector.tensor_copy(out=pt_sb[:], in_=pt[:])
                        ps2 = pm.tile([128, hid], F32, tag="ps2")
                        nc.tensor.matmul(
                            out=ps2[:], lhsT=pt_sb[:], rhs=w1_t[:],
                            start=True, stop=True,
                        )
                        zt = ep.tile([128, hid], F32, tag="zt1")
                        nc.vector.tensor_tensor(
                            out=zt[:], in0=ps2[:], in1=b1_t[:],
                            op=mybir.AluOpType.add,
                        )
                        a1 = ep.tile([128, hid], F16, tag="a1")
                        nc.vector.tensor_scalar_max(
                            out=a1[:], in0=zt[:], scalar1=0.0
                        )
                        nc.sync.dma_start(
                            out=acc1loc[w * 128 : w * 128 + rows, 0:hid],
                            in_=a1[:rows, :],
                        )
                    if post_window is not None:
                        post_window(w)
                    else:
                        # out[w] = log_softmax(P2T.T @ W2 + b2)
                        p2_sb = ep.tile([hid, 128], F16, tag="p2sb")
                        nc.vector.tensor_copy(out=p2_sb[:], in_=pt[:])
                        pso = pm.tile([128, out_dim], F32, tag="pso")
                        nc.tensor.matmul(
                            out=pso[:], lhsT=p2_sb[:], rhs=w2_t[:],
                            start=True, stop=True,
                        )
                        zt = ep.tile([128, out_dim], F32, tag="zt2")
                        nc.vector.tensor_tensor(
                            out=zt[:], in0=pso[:], in1=b2_t[:],
                            op=mybir.AluOpType.add,
                        )
                        mx = ep.tile([128, 1], F32, tag="mx")
                        nc.vector.reduce_max(
                            mx[:], zt[:], axis=mybir.AxisListType.X
                        )
                        sh = ep.tile([128, out_dim], F32, tag="sh")
                        nc.vector.tensor_tensor(
                            out=sh[:], in0=zt[:],
                            in1=mx[:].broadcast_to([128, out_dim]),
                            op=mybir.AluOpType.subtract,
                        )
                        ex = ep.tile([128, out_dim], F32, tag="ex")
                        sm = ep.tile([128, 1], F32, tag="sm")
                        nc.scalar.activation(
                            out=ex[:], in_=sh[:],
                            func=mybir.ActivationFunctionType.Exp,
                            accum_out=sm[:],
                        )
                        lnt = ep.tile([128, 1], F32, tag="lnt")
                        nc.scalar.activation(
                            out=lnt[:], in_=sm[:],
                            func=mybir.ActivationFunctionType.Ln,
                        )
                        res = ep.tile([128, out_dim], F32, tag="res")
                        nc.vector.tensor_tensor(
                            out=res[:], in0=sh[:],
                            in1=lnt[:].broadcast_to([128, out_dim]),
                            op=mybir.AluOpType.subtract,
                        )
                        nc.sync.dma_start(
                            out=out[w * 128 : w * 128 + rows, :],
                            in_=res[:rows, :],
                        )

            # ---- pipeline ----
            for rnd in range(rounds):
                rows2 = s2["rng"]
                h2vs = [
                    dp.tile(
                        [rows2, 2 * hid], F16, addr_space="Shared",
                        tag=f"h2v{rnd}_{k}", name=f"h2v{rnd}_{k}",
                    )
                    for k in range(AGC)
                ]
                # chunked AllGather: chunk k moves each core's acc1 rows
                # [k*nhalf, (k+1)*nhalf) into its own table tensor.  All but
                # the last chunk are emitted mid-agg1, a GBUFS-lookahead past
                # the last contributing window, so the Pool SEQ reaches them
                # right as their input windows land and the collective runs
                # hidden under agg1's tail.  The last chunk is emitted after
                # a pre-issued batch of next-layer range-0 gathers, which
                # then transfer concurrently with it.
                nhalf = nloc // AGC
                emit_w = {}
                for k in range(AGC - 1):
                    bw = math.ceil((k + 1) * nhalf / 128) - 1 + 4
                    emit_w[min(wn - 1, bw)] = k

                def emit_ag(k):
                    nc.gpsimd.collective_compute(
                        "AllGather",
                        mybir.AluOpType.bypass,
                        replica_groups=[list(range(n_cores))],
                        ins=[acc1loc[k * nhalf : (k + 1) * nhalf, :].opt()],
                        outs=[h2vs[k][:].opt()],
                    )

                def post_w(w):
                    if w in emit_w:
                        emit_ag(emit_w[w])

                agg_pass(s1, [xtab], n_nodes, idx1_t, drel1_t, nrm1_t,
                         nneg1_t, layer=1, post_window=post_w)

                def pre2(ensure):
                    # final AG first so its dispatch isn't gated behind the
                    # prefetch batch; then prefetch range-0 gathers, which
                    # transfer concurrently with the collective.  Leave >=2
                    # buffers for the other queues' first chunks, or window 0
                    # deadlocks waiting on range 1.
                    emit_ag(AGC - 1)
                    t0 = s2["t_start"][0]
                    t1 = s2["t_start"][1]
                    ensure(0, min((GBUFS - 2) * CHUNK, t1 - t0))

                agg_pass(s2, h2vs, npair, idx2_t, drel2_t, nrm2_t, nneg2_t,
                         layer=2, pre_hook=pre2)

    _split_long_waits(nc)
    mybir.codegen_inst_isa_subclasses(nc)
    return nc


def _prepare(x, edge_index, W1, b1, W2, b2, n_nodes=N_NODES, n_cores=N_CORES):
    x = np.asarray(x, dtype=np.float32)
    W1 = np.asarray(W1, dtype=np.float32)
    b1 = np.asarray(b1, dtype=np.float32)
    W2 = np.asarray(W2, dtype=np.float32)
    b2 = np.asarray(b2, dtype=np.float32)

    in_dim = x.shape[1]
    hid = W1.shape[1]
    out_dim = W2.shape[1]

    meta = _preprocess(edge_index, n_nodes, n_cores)

    nc = _build_nc(meta, n_nodes, hid, out_dim, in_dim, n_cores)

    xtab = np.ascontiguousarray(x.astype(np.float16))
    w1h = W1.astype(np.float16)
    w2h = W2.astype(np.float16)
    b1rep = np.tile(b1[None, :], (128, 1)).astype(np.float32)
    b2rep = np.tile(b2[None, :], (128, 1)).astype(np.float32)
    iota = np.tile(np.arange(128, dtype=np.float16)[None, :], (128, 1)).copy()
    iotap = np.arange(128, dtype=np.float32).reshape(128, 1).copy()

    nloc = meta["nloc"]
    wn = meta["wn"]
    npad = wn * 128
    s1, s2 = meta["s1"], meta["s2"]
    in_maps = []
    for c in range(n_cores):
        xs = np.zeros((npad, in_dim), dtype=np.float16)
        xs[:nloc] = xtab[c * nloc : (c + 1) * nloc]
        xloc = np.ascontiguousarray(
            xs.reshape(wn, 128, in_dim).transpose(1, 0, 2).reshape(128, npad)
        )
        in_maps.append(
            {
                "xtab": xtab,
                "xloc": xloc,
                "dinvw": meta["dinvw"][c],
                "iotap": iotap,
                "idx1": s1["idx"][c],
                "drel1": s1["drel"][c],
                "nrm1": s1["nrm"][c],
                "idx2": s2["idx"][c],
                "drel2": s2["drel"][c],
                "nrm2": s2["nrm"][c],
                "w1": w1h,
                "w2": w2h,
                "b1rep": b1rep,
                "b2rep": b2rep,
                "iota": iota,
            }
        )
    return nc, in_maps


def kernel(x, edge_index, W1, b1, W2, b2):
    nc, in_maps = _prepare(x, edge_index, W1, b1, W2, b2)
    res = run_bass_kernel_spmd(nc, in_maps, core_ids=list(range(N_CORES)))
    return np.concatenate([res.results[c]["out"] for c in range(N_CORES)], axis=0)


# revision 44
# speedup vs baseline: 1.2393x; 1.2393x over previous
"""Two-layer GCN (PyG GCNConv semantics) on 8 Trainium2 NeuronCores.

Strategy: nodes are sharded row-wise across the 8 cores; edges are
partitioned by destination node so the segment-sum stays local.

Layer 1 gathers raw x rows (128 f16 = 256B, the SWDGE minimum, fully
used) directly from a replicated DRAM table and folds W1 in after
aggregation: PT[in, dst] = sum_e x[src_e] * S, then acc1 = relu(PT.T @ W1
+ b1).  This removes both the first pre-matmul and the first (25.6MB)
AllGather from the critical path; layer-1 gathers start at t=0.

Layer 2 all-gathers the compact acc1 [nloc, 64] f16 (12.8MB full) viewed
as a [50000, 128] pair table (256B rows hold two nodes).  Edges are
sorted by (dst window, pair range, src parity) so each 128-edge tile
statically consumes one 64-wide half of its gathered rows; W2 is folded
in per window after aggregation, then bias + log_softmax.

Gather pipeline: batched SWDGE dma_gather (int16 indices, CHUNK*128 rows
per instruction), one queue per index range.  Aggregation per 128-node
destination window: a norm-weighted one-hot S[e, j] = norm[e] *
(dst_rel[e] == j) built in one DVE tensor_scalar per tile (1 in 6 on the
scalar engine), PE matmuls accumulate into PSUM.

Edge bookkeeping (sorting, slot assignment, padding so all 8 cores share
one instruction stream) is host-side numpy index work; all floating-point
math on features runs on device.
"""

import math

import numpy as np

import concourse.bass as bass
import concourse.mybir as mybir
import concourse.tile as tile
from concourse import library_config
from concourse.bass_utils import run_bass_kernel_spmd

N_NODES = 100000
N_EDGES = 1600000
IN_DIM, HID_DIM, OUT_DIM = 128, 64, 40
N_CORES = 8

RNG = 25000          # table rows per gather range (int16 index limit)
CHUNK = 32           # tiles per dma_gather instruction
GBUFS = 8            # gather chunk buffers in flight
SBUFS = 16
ACT_NTH = 0          # if >0, every ACT_NTH-th S-build goes to scalar engine
PL_NTH = 0           # if >0, every PL_NTH-th (offset 1) S-build on gpsimd
SB_IOTA = False      # S-build reads iota from SBUF f16 (fast DVE mode) vs PSUM
AGC = 2              # AllGather chunks (table row order is chunk-major)

F32 = mybir.dt.float32
F16 = mybir.dt.float16
I16 = mybir.dt.int16


def _split_long_waits(nc, max_waits=1):
    """This toolchain's codegen rejects instructions carrying more than one
    semaphore wait; move extra waits onto preceding same-engine no-ops."""
    cnt = 0
    for bb in nc.main_func.blocks:
        i = 0
        insts = bb.instructions
        while i < len(insts):
            ins = insts[i]
            si = ins.sync_info
            if si is not None and si.on_wait and len(si.on_wait) > max_waits:
                waits = list(si.on_wait)
                keep = waits[-max_waits:]
                extra = waits[:-max_waits]
                si.on_wait = keep
                new_insts = []
                for j in range(0, len(extra), max_waits):
                    chunk = extra[j : j + max_waits]
                    nop = mybir.InstNoOp(
                        name=f"{ins.name}-waitsplit-{j}",
                        engine=ins.engine,
                        ins=[],
                        outs=[],
                        sync_info=mybir.SyncInfo(on_wait=chunk, on_update=[]),
                    )
                    new_insts.append(nop)
                insts[i:i] = new_insts
                i += len(new_insts)
                cnt += len(new_insts)
            i += 1
    return cnt


def _mkstream(n_cores, wn, core, w, gsub, n_gsub, queue_of_gsub, nq,
              idxval, drel, nrm):
    """Group edges by (core, dst window, subgroup); lay out 128-slot tiles
    shared across cores (single SPMD program).  Tile ids are queue-major;
    within a queue they follow consumption order (w, then subgroup, then
    k), so per-queue gathers are contiguous slices.

    Returns device-layout index/scalar arrays plus the tile layout.
    """
    key = (core.astype(np.int64) * wn + w) * n_gsub + gsub
    order = np.argsort(key, kind="stable")
    counts = np.bincount(key, minlength=n_cores * wn * n_gsub).reshape(
        n_cores, wn, n_gsub
    )
    k_wg = (counts.max(axis=0) + 127) // 128  # [wn, n_gsub]

    gs_of_q = [[g for g in range(n_gsub) if queue_of_gsub[g] == q]
               for q in range(nq)]
    tile_first = np.zeros((wn, n_gsub), dtype=np.int64)
    t = 0
    t_start = []
    for q in range(nq):
        t_start.append(t)
        for ww in range(wn):
            for g in gs_of_q[q]:
                tile_first[ww, g] = t
                t += int(k_wg[ww, g])
    T = t
    t_start.append(T)

    grp_start = np.zeros(n_cores * wn * n_gsub, dtype=np.int64)
    grp_start[1:] = np.cumsum(counts.reshape(-1))[:-1]
    rank = np.arange(len(order), dtype=np.int64) - grp_start[key[order]]

    w_o = w[order]
    g_o = gsub[order]
    slot = (tile_first[w_o, g_o] + (rank >> 7)) * 128 + (rank & 127)

    n_slots = T * 128
    idx_a = np.zeros((n_cores, n_slots), dtype=np.int16)
    drel_a = np.zeros((n_cores, n_slots), dtype=np.float32)
    nrm_a = np.zeros((n_cores, n_slots), dtype=np.float32)
    c_o = core[order]
    idx_a[c_o, slot] = idxval[order].astype(np.int16)
    drel_a[c_o, slot] = drel[order]
    nrm_a[c_o, slot] = nrm[order]

    # device layouts:
    #   idx: [128, T*8] int16, slot s at [16g + s%16, s//16], g=0..7
    #   drel/nrm: [128, T] f32, slot s at [s%128, s//128]
    idx_dev, drel_dev, nrm_dev = [], [], []
    for c in range(n_cores):
        base = idx_a[c].reshape(n_slots // 16, 16).T
        idx_dev.append(np.tile(base, (8, 1)).copy())
        drel_dev.append(drel_a[c].reshape(T, 128).T.copy())
        nrm_dev.append(nrm_a[c].reshape(T, 128).T.copy())

    return {
        "T": T,
        "t_start": t_start,
        "nq": nq,
        "k_wg": k_wg,
        "tile_first": tile_first,
        "gs_of_q": gs_of_q,
        "queue_of_gsub": list(queue_of_gsub),
        "idx": idx_dev,
        "drel": drel_dev,
        "nrm": nrm_dev,
        # host-side slot arrays for preprocessing verification
        "host_slot": (c_o, slot, order),
    }


def _preprocess(edge_index, n_nodes, n_cores):
    nloc = n_nodes // n_cores
    wn = math.ceil(nloc / 128)

    src = np.asarray(edge_index[0], dtype=np.int64)
    dst = np.asarray(edge_index[1], dtype=np.int64)
    loop = np.arange(n_nodes, dtype=np.int64)
    src_all = np.concatenate([src, loop])
    dst_all = np.concatenate([dst, loop])

    deg = np.bincount(dst_all, minlength=n_nodes).astype(np.float64)
    dis = np.where(deg > 0, 1.0 / np.sqrt(deg), 0.0)
    norm = (dis[src_all] * dis[dst_all]).astype(np.float32)

    core = dst_all // nloc
    dloc = dst_all - core * nloc
    w = dloc >> 7
    drel = (dloc & 127).astype(np.float32)

    # layer 1: gather x rows; ranges of RNG node rows.  Self-loops are
    # excluded — they are fed from the resident local x window on device
    # (their diagonal S carries dinv = 1/deg).
    ne = len(src)
    r1 = src // RNG
    nr1 = int(src_all.max()) // RNG + 1
    s1 = _mkstream(
        n_cores, wn, core[:ne], w[:ne],
        gsub=r1, n_gsub=nr1, queue_of_gsub=list(range(nr1)), nq=nr1,
        idxval=src - r1 * RNG, drel=drel[:ne], nrm=norm[:ne],
    )
    s1["par"] = None
    s1["rng"] = RNG

    # per-window dinv columns for the layer-1 self-loop diagonal tiles
    dinv = (dis * dis).astype(np.float32)  # 1/deg
    npad = wn * 128
    dinvw = []
    for c in range(n_cores):
        dpad = np.zeros(npad, dtype=np.float32)
        dpad[:nloc] = dinv[c * nloc : (c + 1) * nloc]
        dinvw.append(dpad.reshape(wn, 128).T.copy())  # [128, wn]

    # layer 2: gather acc1 pair rows (two 64-wide nodes per 256B row).
    # Table row order is AllGather-chunk-major: chunk k holds pair-rows
    # [k*PHALF, (k+1)*PHALF) of every core, so each of the AGC collectives
    # reads/writes a contiguous block and chunk == gather range.
    # subgroup = (chunk, parity) so tiles are single-half.
    phalf = nloc // 2 // AGC
    sc = src_all // nloc          # core of source node
    sll = (src_all % nloc) >> 1   # local pair row
    k2 = sll // phalf             # AG chunk
    row2 = k2 * (n_cores * phalf) + sc * phalf + (sll - k2 * phalf)
    assert n_cores * phalf <= RNG
    par = (src_all & 1).astype(np.int64)
    g2 = k2 * 2 + par
    s2 = _mkstream(
        n_cores, wn, core, w,
        gsub=g2, n_gsub=AGC * 2,
        queue_of_gsub=[g // 2 for g in range(AGC * 2)], nq=AGC,
        idxval=row2 - k2 * (n_cores * phalf), drel=drel, nrm=norm,
    )
    s2["par"] = [g % 2 for g in range(AGC * 2)]  # parity per subgroup
    s2["rng"] = n_cores * phalf   # table rows per range (= per AG chunk)

    return {
        "nloc": nloc,
        "wn": wn,
        "s1": s1,
        "s2": s2,
        "dinvw": dinvw,
        "src_all": src_all,
        "dst_all": dst_all,
        "norm_all": norm,
    }


def _build_nc(meta, n_nodes, hid, out_dim, in_dim, n_cores, rounds=1):
    nloc = meta["nloc"]
    wn = meta["wn"]
    s1, s2 = meta["s1"], meta["s2"]
    npair = n_nodes // 2

    nc = bass.Bass(num_devices=n_cores, num_swdge_queues=4)

    xtab = nc.dram_tensor("xtab", [n_nodes, in_dim], F16, kind="ExternalInput")
    xloc = nc.dram_tensor("xloc", [128, wn * 128], F16, kind="ExternalInput")
    dinvw = nc.dram_tensor("dinvw", [128, wn], F32, kind="ExternalInput")
    iotap_in = nc.dram_tensor("iotap", [128, 1], F32, kind="ExternalInput")
    idx1 = nc.dram_tensor("idx1", [128, s1["T"] * 8], I16, kind="ExternalInput")
    drel1 = nc.dram_tensor("drel1", [128, s1["T"]], F32, kind="ExternalInput")
    nrm1 = nc.dram_tensor("nrm1", [128, s1["T"]], F32, kind="ExternalInput")
    idx2 = nc.dram_tensor("idx2", [128, s2["T"] * 8], I16, kind="ExternalInput")
    drel2 = nc.dram_tensor("drel2", [128, s2["T"]], F32, kind="ExternalInput")
    nrm2 = nc.dram_tensor("nrm2", [128, s2["T"]], F32, kind="ExternalInput")
    w1 = nc.dram_tensor("w1", [in_dim, hid], F16, kind="ExternalInput")
    w2 = nc.dram_tensor("w2", [hid, out_dim], F16, kind="ExternalInput")
    b1rep = nc.dram_tensor("b1rep", [128, hid], F32, kind="ExternalInput")
    b2rep = nc.dram_tensor("b2rep", [128, out_dim], F32, kind="ExternalInput")
    iota_in = nc.dram_tensor("iota", [128, 128], F16, kind="ExternalInput")
    out = nc.dram_tensor("out", [nloc, out_dim], F32, kind="ExternalOutput")

    eq = mybir.AluOpType.is_equal
    mul = mybir.AluOpType.mult

    with tile.TileContext(nc) as tc:
        with (
            tc.tile_pool(name="const", bufs=1) as cp,
            tc.tile_pool(name="gpool", bufs=GBUFS) as gp,
            tc.tile_pool(name="spool", bufs=SBUFS) as sp,
            tc.tile_pool(name="evac", bufs=6) as ep,
            tc.tile_pool(name="ps_agg", bufs=3, space="PSUM") as pa,
            tc.tile_pool(name="ps_mm", bufs=2, space="PSUM") as pm,
            tc.tile_pool(name="ps_const", bufs=1, space="PSUM") as pc,
            tc.tile_pool(name="dram", bufs=1, space="DRAM") as dp,
        ):
            nc.gpsimd.load_library(library_config.mlp)

            # ---- resident tensors ----
            def load(name, dram, shape, dt):
                t = cp.tile(shape, dt, name=name)
                nc.sync.dma_start(out=t[:], in_=dram[:])
                return t

            idx1_t = load("idx1t", idx1, [128, s1["T"] * 8], I16)
            drel1_t = load("drel1t", drel1, [128, s1["T"]], F32)
            nrm1_t = load("nrm1t", nrm1, [128, s1["T"]], F32)
            idx2_t = load("idx2t", idx2, [128, s2["T"] * 8], I16)
            drel2_t = load("drel2t", drel2, [128, s2["T"]], F32)
            nrm2_t = load("nrm2t", nrm2, [128, s2["T"]], F32)
            w1_t = load("w1t", w1, [in_dim, hid], F16)
            w2_t = load("w2t", w2, [hid, out_dim], F16)
            b1_t = load("b1t", b1rep, [128, hid], F32)
            b2_t = load("b2t", b2rep, [128, out_dim], F32)
            iota_t = load("iotat", iota_in, [128, 128], F16)
            xloc_t = load("xloct", xloc, [128, wn * 128], F16)
            dinvw_t = load("dinvwt", dinvw, [128, wn], F32)
            iotap_t = load("iotapt", iotap_in, [128, 1], F32)

            if ACT_NTH:
                # negated norms for the ACT-path S-build (scale = -nrm)
                nneg1_t = cp.tile([128, s1["T"]], F32, name="nneg1t")
                nc.vector.tensor_scalar_mul(
                    out=nneg1_t[:], in0=nrm1_t[:], scalar1=-1.0
                )
                nneg2_t = cp.tile([128, s2["T"]], F32, name="nneg2t")
                nc.vector.tensor_scalar_mul(
                    out=nneg2_t[:], in0=nrm2_t[:], scalar1=-1.0
                )
            else:
                nneg1_t = nneg2_t = None

            if SB_IOTA:
                # all-SBUF f16 operands let the DVE S-build use its fast
                # perf mode (risk: 2-port mode contends with GPSIMD SBUF
                # access for SWDGE descriptor rings)
                iota_s = iota_t
            else:
                # iota in PSUM keeps the DVE S-builds out of 2-port perf mode
                iota_ps = pc.tile([128, 128], F32)
                nc.scalar.activation(
                    out=iota_ps[:], in_=iota_t[:],
                    func=mybir.ActivationFunctionType.Identity,
                )
                iota_s = iota_ps

            acc1loc = dp.tile([nloc, hid], F16, name="acc1loc")

            nidx_regs = {}

            def nidx_reg(n):
                if n not in nidx_regs:
                    nidx_regs[n] = nc.gpsimd.to_reg(n)
                return nidx_regs[n]

            # SWDGE sem lanes (8) are assigned round-robin per Pool-DMA
            # instruction and each lane is locked to one hardware queue, so
            # rotate queues with the emission counter to keep lane->queue
            # mapping consistent.
            gq_counter = [0]

            def next_queue():
                q = gq_counter[0] % 4
                gq_counter[0] += 1
                return q

            def build_s(st, t, drel_t, nrm_t, nneg_t):
                """S[e, j] = norm[e] * (dst_rel[e] == j)."""
                s = sp.tile([128, 128], F16, tag="s", name="s")
                if ACT_NTH and t % ACT_NTH == ACT_NTH - 1:
                    # ACT path: relu(norm - norm*(drel-iota)^2) == norm iff eq
                    tmp = sp.tile([128, 128], F16, tag="stmp", name="stmp")
                    nc.scalar.activation(
                        out=tmp[:], in_=iota_t[:],
                        func=mybir.ActivationFunctionType.Square,
                        bias=drel_t[:, t : t + 1], scale=-1.0,
                    )
                    nc.scalar.activation(
                        out=s[:], in_=tmp[:],
                        func=mybir.ActivationFunctionType.Relu,
                        bias=nrm_t[:, t : t + 1], scale=nneg_t[:, t : t + 1],
                    )
                else:
                    eng = nc.vector
                    if PL_NTH and t % PL_NTH == 1:
                        eng = nc.gpsimd
                    eng.tensor_scalar(
                        out=s[:], in0=iota_s[:],
                        scalar1=drel_t[:, t : t + 1],
                        scalar2=nrm_t[:, t : t + 1],
                        op0=eq, op1=mul,
                    )
                return s

            def agg_pass(st, tables, table_rows, idx_t, drel_t, nrm_t, nneg_t,
                         layer, post_window=None, pre_hook=None):
                """Chunk-pipelined gather + per-window aggregation."""
                nq = st["nq"]
                t_start = st["t_start"]
                k_wg = st["k_wg"]
                tile_first = st["tile_first"]
                par_of = st["par"]
                issued = [0] * nq
                chunks = [[] for _ in range(nq)]

                rng = st["rng"]

                def ensure(q, tiles_needed):
                    T_q = t_start[q + 1] - t_start[q]
                    while issued[q] < tiles_needed:
                        t0 = t_start[q] + issued[q]
                        span = min(CHUNK, T_q - issued[q])
                        g = gp.tile([128, CHUNK, 128], F16, tag="g", name="g")
                        if len(tables) == 1:  # one table, per-range offsets
                            rows0 = q * rng
                            rows1 = min(rows0 + rng, table_rows)
                            tab = tables[0][rows0:rows1, :]
                        else:  # one table per range
                            tab = tables[q][:, :]
                        nc.gpsimd.dma_gather(
                            g[:, 0:span, :],
                            tab,
                            idx_t[:, t0 * 8 : (t0 + span) * 8],
                            span * 128,
                            nidx_reg(span * 128),
                            128,
                            single_packet=False,
                            queue_num=next_queue(),
                        )
                        chunks[q].append((g, t0, span))
                        issued[q] += span

                def gview(q, t):
                    while True:
                        g, t0, span = chunks[q][0]
                        if t < t0 + span:
                            return g[:, t - t0, :]
                        chunks[q].pop(0)

                if pre_hook is not None:
                    pre_hook(ensure)

                for w in range(wn):
                    # consumption order: subgroups by (queue, in-queue order)
                    todo = []  # (t, q, par)
                    ktot = 1 if layer == 1 else 0  # self-tile counts in L1
                    for q in range(nq):
                        for g in st["gs_of_q"][q]:
                            kw = int(k_wg[w][g])
                            if kw == 0:
                                continue
                            ensure(q, tile_first[w][g] - t_start[q] + kw)
                            p = 0 if par_of is None else par_of[g]
                            for k in range(kw):
                                todo.append((int(tile_first[w][g]) + k, q, p))
                            ktot += kw
                    if layer == 1:
                        pt = pa.tile([in_dim, 128], F32, tag="pagg")
                        # self-loop diagonal tile from the resident local x
                        # window: S_self = (iota == p) * dinv
                        s_self = sp.tile([128, 128], F16, tag="s", name="s")
                        nc.vector.tensor_scalar(
                            out=s_self[:], in0=iota_s[:],
                            scalar1=iotap_t[:],
                            scalar2=dinvw_t[:, w : w + 1],
                            op0=eq, op1=mul,
                        )
                        nc.tensor.matmul(
                            out=pt[:],
                            lhsT=xloc_t[:, w * 128 : (w + 1) * 128],
                            rhs=s_self[:],
                            start=True, stop=(ktot == 1),
                        )
                        ki0 = 1
                    else:
                        pt = pa.tile([hid, 128], F32, tag="pagg")
                        ki0 = 0
                    for ki, (t, q, p) in enumerate(todo, start=ki0):
                        gfull = gview(q, t)
                        if layer == 1:
                            gv = gfull[:, 0:in_dim]
                        else:
                            gv = gfull[:, p * hid : (p + 1) * hid]
                        s = build_s(st, t, drel_t, nrm_t, nneg_t)
                        nc.tensor.matmul(
                            out=pt[:], lhsT=gv, rhs=s[:],
                            start=(ki == 0), stop=(ki == ktot - 1),
                        )
                    rows = min(128, nloc - w * 128)
                    if layer == 1:
                        # acc1[w] = relu(PT.T @ W1 + b1)
                        pt_sb = ep.tile([in_dim, 128], F16, tag="ptsb")
                        nc.vector.tensor_copy(out=pt_sb[:], in_=pt[:])
                        ps2 = pm.tile([128, hid], F32, tag="ps2")
                        nc.tensor.matmul(
                            out=ps2[:], lhsT=pt_sb[:], rhs=w1_t[:],
                            start=True, stop=True,
                        )
                        zt = ep.tile([128, hid], F32, tag="zt1")
                        nc.vector.tensor_tensor(
                            out=zt[:], in0=ps2[:], in1=b1_t[:],
                            op=mybir.AluOpType.add,
                        )
                        a1 = ep.tile([128, hid], F16, tag="a1")
                        nc.vector.tensor_scalar_max(
                            out=a1[:], in0=zt[:], scalar1=0.0
                        )
                        nc.sync.dma_start(
                            out=acc1loc[w * 128 : w * 128 + rows, 0:hid],
                            in_=a1[:rows, :],
                        )
                    if post_window is not None:
                        post_window(w)
                    else:
                        # out[w] = log_softmax(P2T.T @ W2 + b2)
                        p2_sb = ep.tile([hid, 128], F16, tag="p2sb")
                        nc.vector.tensor_copy(out=p2_sb[:], in_=pt[:])
                        pso = pm.tile([128, out_dim], F32, tag="pso")
                        nc.tensor.matmul(
                            out=pso[:], lhsT=p2_sb[:], rhs=w2_t[:],
                            start=True, stop=True,
                        )
                        zt = ep.tile([128, out_dim], F32, tag="zt2")
                        nc.vector.tensor_tensor(
                            out=zt[:], in0=pso[:], in1=b2_t[:],
                            op=mybir.AluOpType.add,
                        )
                        mx = ep.tile([128, 1], F32, tag="mx")
                        nc.vector.reduce_max(
                            mx[:], zt[:], axis=mybir.AxisListType.X
                        )
                        sh = ep.tile([128, out_dim], F32, tag="sh")
                        nc.vector.tensor_tensor(
                            out=sh[:], in0=zt[:],
                            in1=mx[:].broadcast_to([128, out_dim]),
                            op=mybir.AluOpType.subtract,
                        )
                        ex = ep.tile([128, out_dim], F32, tag="ex")
                        sm = ep.tile([128, 1], F32, tag="sm")
                        nc.scalar.activation(
                            out=ex[:], in_=sh[:],
                            func=mybir.ActivationFunctionType.Exp,
                            accum_out=sm[:],
                        )
                        lnt = ep.tile([128, 1], F32, tag="lnt")
                        nc.scalar.activation(
                            out=lnt[:], in_=sm[:],
                            func=mybir.ActivationFunctionType.Ln,
                        )
                        res = ep.tile([128, out_dim], F32, tag="res")
                        nc.vector.tensor_tensor(
                            out=res[:], in0=sh[:],
                            in1=lnt[:].broadcast_to([128, out_dim]),
                            op=mybir.AluOpType.subtract,
                        )
                        nc.sync.dma_start(
                            out=out[w * 128 : w * 128 + rows, :],
                            in_=res[:rows, :],
                        )

            # ---- pipeline ----
            for rnd in range(rounds):
                rows2 = s2["rng"]
                h2vs = [
                    dp.tile(
                        [rows2, 2 * hid], F16, addr_space="Shared",
                        tag=f"h2v{rnd}_{k}", name=f"h2v{rnd}_{k}",
                    )
                    for k in range(AGC)
                ]
                # chunked AllGather: chunk k moves each core's acc1 rows
                # [k*nhalf, (k+1)*nhalf) into its own table tensor.  All but
                # the last chunk are emitted mid-agg1, a GBUFS-lookahead past
                # the last contributing window, so the Pool SEQ reaches them
                # right as their input windows land and the collective runs
                # hidden under agg1's tail.  The last chunk is emitted after
                # a pre-issued batch of next-layer range-0 gathers, which
                # then transfer concurrently with it.
                nhalf = nloc // AGC
                emit_w = {}
                for k in range(AGC - 1):
                    bw = math.ceil((k + 1) * nhalf / 128) - 1 + 4
                    emit_w[min(wn - 1, bw)] = k

                def emit_ag(k):
                    nc.gpsimd.collective_compute(
                        "AllGather",
                        mybir.AluOpType.bypass,
                        replica_groups=[list(range(n_cores))],
                        ins=[acc1loc[k * nhalf : (k + 1) * nhalf, :].opt()],
                        outs=[h2vs[k][:].opt()],
                    )

                def post_w(w):
                    if w in emit_w:
                        emit_ag(emit_w[w])

                agg_pass(s1, [xtab], n_nodes, idx1_t, drel1_t, nrm1_t,
                         nneg1_t, layer=1, post_window=post_w)

                def pre2(ensure):
                    # final AG first so its dispatch isn't gated behind the
                    # prefetch batch; then prefetch range-0 gathers, which
                    # transfer concurrently with the collective.  Leave >=2
                    # buffers for the other queues' first chunks, or window 0
                    # deadlocks waiting on range 1.
                    emit_ag(AGC - 1)
                    t0 = s2["t_start"][0]
                    t1 = s2["t_start"][1]
                    ensure(0, min((GBUFS - 2) * CHUNK, t1 - t0))

                agg_pass(s2, h2vs, npair, idx2_t, drel2_t, nrm2_t, nneg2_t,
                         layer=2, pre_hook=pre2)

    _split_long_waits(nc)
    mybir.codegen_inst_isa_subclasses(nc)
    return nc


def _prepare(x, edge_index, W1, b1, W2, b2, n_nodes=N_NODES, n_cores=N_CORES):
    x = np.asarray(x, dtype=np.float32)
    W1 = np.asarray(W1, dtype=np.float32)
    b1 = np.asarray(b1, dtype=np.float32)
    W2 = np.asarray(W2, dtype=np.float32)
    b2 = np.asarray(b2, dtype=np.float32)

    in_dim = x.shape[1]
    hid = W1.shape[1]
    out_dim = W2.shape[1]

    meta = _preprocess(edge_index, n_nodes, n_cores)

    nc = _build_nc(meta, n_nodes, hid, out_dim, in_dim, n_cores)

    xtab = np.ascontiguousarray(x.astype(np.float16))
    w1h = W1.astype(np.float16)
    w2h = W2.astype(np.float16)
    b1rep = np.tile(b1[None, :], (128, 1)).astype(np.float32)
    b2rep = np.tile(b2[None, :], (128, 1)).astype(np.float32)
    iota = np.tile(np.arange(128, dtype=np.float16)[None, :], (128, 1)).copy()
    iotap = np.arange(128, dtype=np.float32).reshape(128, 1).copy()

    nloc = meta["nloc"]
    wn = meta["wn"]
    npad = wn * 128
    s1, s2 = meta["s1"], meta["s2"]
    in_maps = []
    for c in range(n_cores):
        xs = np.zeros((npad, in_dim), dtype=np.float16)
        xs[:nloc] = xtab[c * nloc : (c + 1) * nloc]
        xloc = np.ascontiguousarray(
            xs.reshape(wn, 128, in_dim).transpose(1, 0, 2).reshape(128, npad)
        )
        in_maps.append(
            {
                "xtab": xtab,
                "xloc": xloc,
                "dinvw": meta["dinvw"][c],
                "iotap": iotap,
                "idx1": s1["idx"][c],
                "drel1": s1["drel"][c],
                "nrm1": s1["nrm"][c],
                "idx2": s2["idx"][c],
                "drel2": s2["drel"][c],
                "nrm2": s2["nrm"][c],
                "w1": w1h,
                "w2": w2h,
                "b1rep": b1rep,
                "b2rep": b2rep,
                "iota": iota,
            }
        )
    return nc, in_maps


def kernel(x, edge_index, W1, b1, W2, b2):
    nc, in_maps = _prepare(x, edge_index, W1, b1, W2, b2)
    res = run_bass_kernel_spmd(nc, in_maps, core_ids=list(range(N_CORES)))
    return np.concatenate([res.results[c]["out"] for c in range(N_CORES)], axis=0)


# revision 46
# speedup vs baseline: 1.3731x; 1.1080x over previous
"""Two-layer GCN (PyG GCNConv semantics) on 8 Trainium2 NeuronCores.

Strategy: nodes are sharded row-wise across the 8 cores; edges are
partitioned by destination node so the segment-sum stays local.

Layer 1 gathers raw x rows (128 f16 = 256B, the SWDGE minimum, fully
used) directly from a replicated DRAM table and folds W1 in after
aggregation: PT[in, dst] = sum_e x[src_e] * S, then acc1 = relu(PT.T @ W1
+ b1).  This removes both the first pre-matmul and the first (25.6MB)
AllGather from the critical path; layer-1 gathers start at t=0.

Layer 2 all-gathers the compact acc1 [nloc, 64] f16 (12.8MB full) viewed
as a [50000, 128] pair table (256B rows hold two nodes).  Edges are
sorted by (dst window, pair range, src parity) so each 128-edge tile
statically consumes one 64-wide half of its gathered rows; W2 is folded
in per window after aggregation, then bias + log_softmax.

Gather pipeline: batched SWDGE dma_gather (int16 indices, CHUNK*128 rows
per instruction), one queue per index range.  Aggregation per 128-node
destination window: a norm-weighted one-hot S[e, j] = norm[e] *
(dst_rel[e] == j) built in one DVE tensor_scalar per tile (1 in 6 on the
scalar engine), PE matmuls accumulate into PSUM.

Edge bookkeeping (sorting, slot assignment, padding so all 8 cores share
one instruction stream) is host-side numpy index work; all floating-point
math on features runs on device.
"""

import math

import numpy as np

import concourse.bass as bass
import concourse.mybir as mybir
import concourse.tile as tile
from concourse import library_config
from concourse.bass_utils import run_bass_kernel_spmd

N_NODES = 100000
N_EDGES = 1600000
IN_DIM, HID_DIM, OUT_DIM = 128, 64, 40
N_CORES = 8

RNG = 25000          # table rows per gather range (int16 index limit)
CHUNK = 32           # tiles per dma_gather instruction
GBUFS = 8            # gather chunk buffers in flight
SBUFS = 16
ACT_NTH = 0          # if >0, every ACT_NTH-th S-build goes to scalar engine
PL_NTH = 0           # if >0, every PL_NTH-th (offset 1) S-build on gpsimd
SB_IOTA = False      # S-build reads iota from SBUF f16 (fast DVE mode) vs PSUM
AGC = 2              # AllGather chunks (table row order is chunk-major)

F32 = mybir.dt.float32
F16 = mybir.dt.float16
I16 = mybir.dt.int16


def _split_long_waits(nc, max_waits=1):
    """This toolchain's codegen rejects instructions carrying more than one
    semaphore wait; move extra waits onto preceding same-engine no-ops."""
    cnt = 0
    for bb in nc.main_func.blocks:
        i = 0
        insts = bb.instructions
        while i < len(insts):
            ins = insts[i]
            si = ins.sync_info
            if si is not None and si.on_wait and len(si.on_wait) > max_waits:
                waits = list(si.on_wait)
                keep = waits[-max_waits:]
                extra = waits[:-max_waits]
                si.on_wait = keep
                new_insts = []
                for j in range(0, len(extra), max_waits):
                    chunk = extra[j : j + max_waits]
                    nop = mybir.InstNoOp(
                        name=f"{ins.name}-waitsplit-{j}",
                        engine=ins.engine,
                        ins=[],
                        outs=[],
                        sync_info=mybir.SyncInfo(on_wait=chunk, on_update=[]),
                    )
                    new_insts.append(nop)
                insts[i:i] = new_insts
                i += len(new_insts)
                cnt += len(new_insts)
            i += 1
    return cnt


def _mkstream(n_cores, wn, core, w, gsub, n_gsub, queue_of_gsub, nq,
              idxval, drel, nrm):
    """Group edges by (core, dst window, subgroup); lay out 128-slot tiles
    shared across cores (single SPMD program).  Tile ids are queue-major;
    within a queue they follow consumption order (w, then subgroup, then
    k), so per-queue gathers are contiguous slices.

    Returns device-layout index/scalar arrays plus the tile layout.
    """
    key = (core.astype(np.int64) * wn + w) * n_gsub + gsub
    order = np.argsort(key, kind="stable")
    counts = np.bincount(key, minlength=n_cores * wn * n_gsub).reshape(
        n_cores, wn, n_gsub
    )
    k_wg = (counts.max(axis=0) + 127) // 128  # [wn, n_gsub]

    gs_of_q = [[g for g in range(n_gsub) if queue_of_gsub[g] == q]
               for q in range(nq)]
    tile_first = np.zeros((wn, n_gsub), dtype=np.int64)
    t = 0
    t_start = []
    for q in range(nq):
        t_start.append(t)
        for ww in range(wn):
            for g in gs_of_q[q]:
                tile_first[ww, g] = t
                t += int(k_wg[ww, g])
    T = t
    t_start.append(T)

    grp_start = np.zeros(n_cores * wn * n_gsub, dtype=np.int64)
    grp_start[1:] = np.cumsum(counts.reshape(-1))[:-1]
    rank = np.arange(len(order), dtype=np.int64) - grp_start[key[order]]

    w_o = w[order]
    g_o = gsub[order]
    slot = (tile_first[w_o, g_o] + (rank >> 7)) * 128 + (rank & 127)

    n_slots = T * 128
    idx_a = np.zeros((n_cores, n_slots), dtype=np.int16)
    drel_a = np.zeros((n_cores, n_slots), dtype=np.float32)
    nrm_a = np.zeros((n_cores, n_slots), dtype=np.float32)
    c_o = core[order]
    idx_a[c_o, slot] = idxval[order].astype(np.int16)
    drel_a[c_o, slot] = drel[order]
    nrm_a[c_o, slot] = nrm[order]

    # device layouts:
    #   idx: [128, T*8] int16, slot s at [16g + s%16, s//16], g=0..7
    #   drel/nrm: [128, T] f32, slot s at [s%128, s//128]
    idx_dev, drel_dev, nrm_dev = [], [], []
    for c in range(n_cores):
        base = idx_a[c].reshape(n_slots // 16, 16).T
        idx_dev.append(np.tile(base, (8, 1)).copy())
        drel_dev.append(drel_a[c].reshape(T, 128).T.copy())
        nrm_dev.append(nrm_a[c].reshape(T, 128).T.copy())

    return {
        "T": T,
        "t_start": t_start,
        "nq": nq,
        "k_wg": k_wg,
        "tile_first": tile_first,
        "gs_of_q": gs_of_q,
        "queue_of_gsub": list(queue_of_gsub),
        "idx": idx_dev,
        "drel": drel_dev,
        "nrm": nrm_dev,
        # host-side slot arrays for preprocessing verification
        "host_slot": (c_o, slot, order),
    }


def _preprocess(edge_index, n_nodes, n_cores):
    nloc = n_nodes // n_cores
    wn = math.ceil(nloc / 128)

    src = np.asarray(edge_index[0], dtype=np.int64)
    dst = np.asarray(edge_index[1], dtype=np.int64)
    loop = np.arange(n_nodes, dtype=np.int64)
    src_all = np.concatenate([src, loop])
    dst_all = np.concatenate([dst, loop])

    deg = np.bincount(dst_all, minlength=n_nodes).astype(np.float64)
    dis = np.where(deg > 0, 1.0 / np.sqrt(deg), 0.0)
    norm = (dis[src_all] * dis[dst_all]).astype(np.float32)

    core = dst_all // nloc
    dloc = dst_all - core * nloc
    w = dloc >> 7
    drel = (dloc & 127).astype(np.float32)

    # layer 1: gather x rows; ranges of RNG node rows.  Self-loops are
    # excluded — they are fed from the resident local x window on device
    # (their diagonal S carries dinv = 1/deg).
    ne = len(src)
    r1 = src // RNG
    nr1 = int(src_all.max()) // RNG + 1
    s1 = _mkstream(
        n_cores, wn, core[:ne], w[:ne],
        gsub=r1, n_gsub=nr1, queue_of_gsub=list(range(nr1)), nq=nr1,
        idxval=src - r1 * RNG, drel=drel[:ne], nrm=norm[:ne],
    )
    s1["par"] = None
    s1["rng"] = RNG

    # per-window dinv columns for the layer-1 self-loop diagonal tiles
    dinv = (dis * dis).astype(np.float32)  # 1/deg
    npad = wn * 128
    dinvw = []
    for c in range(n_cores):
        dpad = np.zeros(npad, dtype=np.float32)
        dpad[:nloc] = dinv[c * nloc : (c + 1) * nloc]
        dinvw.append(dpad.reshape(wn, 128).T.copy())  # [128, wn]

    # layer 2: gather acc1 pair rows (two 64-wide nodes per 256B row).
    # Table row order is AllGather-chunk-major: chunk k holds pair-rows
    # [k*PHALF, (k+1)*PHALF) of every core, so each of the AGC collectives
    # reads/writes a contiguous block and chunk == gather range.
    # subgroup = (chunk, parity) so tiles are single-half.
    phalf = nloc // 2 // AGC
    sc = src_all // nloc          # core of source node
    sll = (src_all % nloc) >> 1   # local pair row
    k2 = sll // phalf             # AG chunk
    row2 = k2 * (n_cores * phalf) + sc * phalf + (sll - k2 * phalf)
    assert n_cores * phalf <= RNG
    par = (src_all & 1).astype(np.int64)
    g2 = k2 * 2 + par
    s2 = _mkstream(
        n_cores, wn, core, w,
        gsub=g2, n_gsub=AGC * 2,
        queue_of_gsub=[g // 2 for g in range(AGC * 2)], nq=AGC,
        idxval=row2 - k2 * (n_cores * phalf), drel=drel, nrm=norm,
    )
    s2["par"] = [g % 2 for g in range(AGC * 2)]  # parity per subgroup
    s2["rng"] = n_cores * phalf   # table rows per range (= per AG chunk)

    return {
        "nloc": nloc,
        "wn": wn,
        "s1": s1,
        "s2": s2,
        "dinvw": dinvw,
        "src_all": src_all,
        "dst_all": dst_all,
        "norm_all": norm,
    }


def _build_nc(meta, n_nodes, hid, out_dim, in_dim, n_cores, rounds=1):
    nloc = meta["nloc"]
    wn = meta["wn"]
    s1, s2 = meta["s1"], meta["s2"]
    npair = n_nodes // 2

    nc = bass.Bass(num_devices=n_cores, num_swdge_queues=4)

    xtab = nc.dram_tensor("xtab", [n_nodes, in_dim], F16, kind="ExternalInput")
    xloc = nc.dram_tensor("xloc", [128, wn * 128], F16, kind="ExternalInput")
    dinvw = nc.dram_tensor("dinvw", [128, wn], F32, kind="ExternalInput")
    iotap_in = nc.dram_tensor("iotap", [128, 1], F32, kind="ExternalInput")
    idx1 = nc.dram_tensor("idx1", [128, s1["T"] * 8], I16, kind="ExternalInput")
    drel1 = nc.dram_tensor("drel1", [128, s1["T"]], F32, kind="ExternalInput")
    nrm1 = nc.dram_tensor("nrm1", [128, s1["T"]], F32, kind="ExternalInput")
    idx2 = nc.dram_tensor("idx2", [128, s2["T"] * 8], I16, kind="ExternalInput")
    drel2 = nc.dram_tensor("drel2", [128, s2["T"]], F32, kind="ExternalInput")
    nrm2 = nc.dram_tensor("nrm2", [128, s2["T"]], F32, kind="ExternalInput")
    w1 = nc.dram_tensor("w1", [in_dim, hid], F16, kind="ExternalInput")
    w2 = nc.dram_tensor("w2", [hid, out_dim], F16, kind="ExternalInput")
    b1rep = nc.dram_tensor("b1rep", [128, hid], F32, kind="ExternalInput")
    b2rep = nc.dram_tensor("b2rep", [128, out_dim], F32, kind="ExternalInput")
    iota_in = nc.dram_tensor("iota", [128, 128], F16, kind="ExternalInput")
    out = nc.dram_tensor("out", [nloc, out_dim], F32, kind="ExternalOutput")

    eq = mybir.AluOpType.is_equal
    mul = mybir.AluOpType.mult

    with tile.TileContext(nc) as tc:
        with (
            tc.tile_pool(name="const", bufs=1) as cp,
            tc.tile_pool(name="gpool", bufs=GBUFS) as gp,
            tc.tile_pool(name="spool", bufs=SBUFS) as sp,
            tc.tile_pool(name="evac", bufs=6) as ep,
            tc.tile_pool(name="ps_agg", bufs=3, space="PSUM") as pa,
            tc.tile_pool(name="ps_mm", bufs=2, space="PSUM") as pm,
            tc.tile_pool(name="ps_const", bufs=1, space="PSUM") as pc,
            tc.tile_pool(name="dram", bufs=1, space="DRAM") as dp,
        ):
            nc.gpsimd.load_library(library_config.mlp)

            # ---- resident tensors ----
            def load(name, dram, shape, dt):
                t = cp.tile(shape, dt, name=name)
                nc.sync.dma_start(out=t[:], in_=dram[:])
                return t

            idx1_t = load("idx1t", idx1, [128, s1["T"] * 8], I16)
            drel1_t = load("drel1t", drel1, [128, s1["T"]], F32)
            nrm1_t = load("nrm1t", nrm1, [128, s1["T"]], F32)
            idx2_t = load("idx2t", idx2, [128, s2["T"] * 8], I16)
            drel2_t = load("drel2t", drel2, [128, s2["T"]], F32)
            nrm2_t = load("nrm2t", nrm2, [128, s2["T"]], F32)
            w1_t = load("w1t", w1, [in_dim, hid], F16)
            w2_t = load("w2t", w2, [hid, out_dim], F16)
            b1_t = load("b1t", b1rep, [128, hid], F32)
            b2_t = load("b2t", b2rep, [128, out_dim], F32)
            iota_t = load("iotat", iota_in, [128, 128], F16)
            xloc_t = load("xloct", xloc, [128, wn * 128], F16)
            dinvw_t = load("dinvwt", dinvw, [128, wn], F32)
            iotap_t = load("iotapt", iotap_in, [128, 1], F32)

            if ACT_NTH:
                # negated norms for the ACT-path S-build (scale = -nrm)
                nneg1_t = cp.tile([128, s1["T"]], F32, name="nneg1t")
                nc.vector.tensor_scalar_mul(
                    out=nneg1_t[:], in0=nrm1_t[:], scalar1=-1.0
                )
                nneg2_t = cp.tile([128, s2["T"]], F32, name="nneg2t")
                nc.vector.tensor_scalar_mul(
                    out=nneg2_t[:], in0=nrm2_t[:], scalar1=-1.0
                )
            else:
                nneg1_t = nneg2_t = None

            if SB_IOTA:
                # all-SBUF f16 operands let the DVE S-build use its fast
                # perf mode (risk: 2-port mode contends with GPSIMD SBUF
                # access for SWDGE descriptor rings)
                iota_s = iota_t
            else:
                # iota in PSUM keeps the DVE S-builds out of 2-port perf mode
                iota_ps = pc.tile([128, 128], F32)
                nc.scalar.activation(
                    out=iota_ps[:], in_=iota_t[:],
                    func=mybir.ActivationFunctionType.Identity,
                )
                iota_s = iota_ps

            acc1loc = dp.tile([nloc, hid], F16, name="acc1loc")

            nidx_regs = {}

            def nidx_reg(n):
                if n not in nidx_regs:
                    nidx_regs[n] = nc.gpsimd.to_reg(n)
                return nidx_regs[n]

            # SWDGE sem lanes (8) are assigned round-robin per Pool-DMA
            # instruction and each lane is locked to one hardware queue, so
            # rotate queues with the emission counter to keep lane->queue
            # mapping consistent.
            gq_counter = [0]

            def next_queue():
                q = gq_counter[0] % 4
                gq_counter[0] += 1
                return q

            def build_s(st, t, drel_t, nrm_t, nneg_t):
                """S[e, j] = norm[e] * (dst_rel[e] == j)."""
                s = sp.tile([128, 128], F16, tag="s", name="s")
                if ACT_NTH and t % ACT_NTH == ACT_NTH - 1:
                    # ACT path: relu(norm - norm*(drel-iota)^2) == norm iff eq
                    tmp = sp.tile([128, 128], F16, tag="stmp", name="stmp")
                    nc.scalar.activation(
                        out=tmp[:], in_=iota_t[:],
                        func=mybir.ActivationFunctionType.Square,
                        bias=drel_t[:, t : t + 1], scale=-1.0,
                    )
                    nc.scalar.activation(
                        out=s[:], in_=tmp[:],
                        func=mybir.ActivationFunctionType.Relu,
                        bias=nrm_t[:, t : t + 1], scale=nneg_t[:, t : t + 1],
                    )
                else:
                    eng = nc.vector
                    if PL_NTH and t % PL_NTH == 1:
                        eng = nc.gpsimd
                    eng.tensor_scalar(
                        out=s[:], in0=iota_s[:],
                        scalar1=drel_t[:, t : t + 1],
                        scalar2=nrm_t[:, t : t + 1],
                        op0=eq, op1=mul,
                    )
                return s

            def agg_pass(st, tables, table_rows, idx_t, drel_t, nrm_t, nneg_t,
                         layer, post_window=None, pre_hook=None):
                """Chunk-pipelined gather + per-window aggregation."""
                nq = st["nq"]
                t_start = st["t_start"]
                k_wg = st["k_wg"]
                tile_first = st["tile_first"]
                par_of = st["par"]
                issued = [0] * nq
                chunks = [[] for _ in range(nq)]

                rng = st["rng"]

                def ensure(q, tiles_needed):
                    T_q = t_start[q + 1] - t_start[q]
                    while issued[q] < tiles_needed:
                        t0 = t_start[q] + issued[q]
                        span = min(CHUNK, T_q - issued[q])
                        g = gp.tile([128, CHUNK, 128], F16, tag="g", name="g")
                        if len(tables) == 1:  # one table, per-range offsets
                            rows0 = q * rng
                            rows1 = min(rows0 + rng, table_rows)
                            tab = tables[0][rows0:rows1, :]
                        else:  # one table per range
                            tab = tables[q][:, :]
                        nc.gpsimd.dma_gather(
                            g[:, 0:span, :],
                            tab,
                            idx_t[:, t0 * 8 : (t0 + span) * 8],
                            span * 128,
                            nidx_reg(span * 128),
                            128,
                            single_packet=False,
                            queue_num=next_queue(),
                        )
                        chunks[q].append((g, t0, span))
                        issued[q] += span

                def gview(q, t):
                    while True:
                        g, t0, span = chunks[q][0]
                        if t < t0 + span:
                            return g[:, t - t0, :]
                        chunks[q].pop(0)

                if pre_hook is not None:
                    pre_hook(ensure)

                for w in range(wn):
                    # consumption order: subgroups by (queue, in-queue order)
                    todo = []  # (t, q, par)
                    ktot = 1 if layer == 1 else 0  # self-tile counts in L1
                    for q in range(nq):
                        for g in st["gs_of_q"][q]:
                            kw = int(k_wg[w][g])
                            if kw == 0:
                                continue
                            ensure(q, tile_first[w][g] - t_start[q] + kw)
                            p = 0 if par_of is None else par_of[g]
                            for k in range(kw):
                                todo.append((int(tile_first[w][g]) + k, q, p))
                            ktot += kw
                    if layer == 1:
                        pt = pa.tile([in_dim, 128], F32, tag="pagg")
                        # self-loop diagonal tile from the resident local x
                        # window: S_self = (iota == p) * dinv
                        s_self = sp.tile([128, 128], F16, tag="s", name="s")
                        nc.vector.tensor_scalar(
                            out=s_self[:], in0=iota_s[:],
                            scalar1=iotap_t[:],
                            scalar2=dinvw_t[:, w : w + 1],
                            op0=eq, op1=mul,
                        )
                        nc.tensor.matmul(
                            out=pt[:],
                            lhsT=xloc_t[:, w * 128 : (w + 1) * 128],
                            rhs=s_self[:],
                            start=True, stop=(ktot == 1),
                        )
                        ki0 = 1
                    else:
                        pt = pa.tile([hid, 128], F32, tag="pagg")
                        ki0 = 0
                    for ki, (t, q, p) in enumerate(todo, start=ki0):
                        gfull = gview(q, t)
                        if layer == 1:
                            gv = gfull[:, 0:in_dim]
                        else:
                            gv = gfull[:, p * hid : (p + 1) * hid]
                        s = build_s(st, t, drel_t, nrm_t, nneg_t)
                        nc.tensor.matmul(
                            out=pt[:], lhsT=gv, rhs=s[:],
                            start=(ki == 0), stop=(ki == ktot - 1),
                        )
                    rows = min(128, nloc - w * 128)
                    if layer == 1:
                        # acc1[w] = relu(PT.T @ W1 + b1)
                        pt_sb = ep.tile([in_dim, 128], F16, tag="ptsb")
                        nc.vector.tensor_copy(out=pt_sb[:], in_=pt[:])
                        ps2 = pm.tile([128, hid], F32, tag="ps2")
                        nc.tensor.matmul(
                            out=ps2[:], lhsT=pt_sb[:], rhs=w1_t[:],
                            start=True, stop=True,
                        )
                        zt = ep.tile([128, hid], F32, tag="zt1")
                        nc.vector.tensor_tensor(
                            out=zt[:], in0=ps2[:], in1=b1_t[:],
                            op=mybir.AluOpType.add,
                        )
                        a1 = ep.tile([128, hid], F16, tag="a1")
                        nc.vector.tensor_scalar_max(
                            out=a1[:], in0=zt[:], scalar1=0.0
                        )
                        nc.sync.dma_start(
                            out=acc1loc[w * 128 : w * 128 + rows, 0:hid],
                            in_=a1[:rows, :],
                        )
                    if post_window is not None:
                        post_window(w)
                    else:
                        # out[w] = log_softmax(P2T.T @ W2 + b2)
                        p2_sb = ep.tile([hid, 128], F16, tag="p2sb")
                        nc.vector.tensor_copy(out=p2_sb[:], in_=pt[:])
                        pso = pm.tile([128, out_dim], F32, tag="pso")
                        nc.tensor.matmul(
                            out=pso[:], lhsT=p2_sb[:], rhs=w2_t[:],
                            start=True, stop=True,
                        )
                        zt = ep.tile([128, out_dim], F32, tag="zt2")
                        nc.vector.tensor_tensor(
                            out=zt[:], in0=pso[:], in1=b2_t[:],
                            op=mybir.AluOpType.add,
                        )
                        mx = ep.tile([128, 1], F32, tag="mx")
                        nc.vector.reduce_max(
                            mx[:], zt[:], axis=mybir.AxisListType.X
                        )
                        sh = ep.tile([128, out_dim], F32, tag="sh")
                        nc.vector.tensor_tensor(
                            out=sh[:], in0=zt[:],
                            in1=mx[:].broadcast_to([128, out_dim]),
                            op=mybir.AluOpType.subtract,
                        )
                        ex = ep.tile([128, out_dim], F32, tag="ex")
                        sm = ep.tile([128, 1], F32, tag="sm")
                        nc.scalar.activation(
                            out=ex[:], in_=sh[:],
                            func=mybir.ActivationFunctionType.Exp,
                            accum_out=sm[:],
                        )
                        lnt = ep.tile([128, 1], F32, tag="lnt")
                        nc.scalar.activation(
                            out=lnt[:], in_=sm[:],
                            func=mybir.ActivationFunctionType.Ln,
                        )
                        res = ep.tile([128, out_dim], F32, tag="res")
                        nc.vector.tensor_tensor(
                            out=res[:], in0=sh[:],
                            in1=lnt[:].broadcast_to([128, out_dim]),
                            op=mybir.AluOpType.subtract,
                        )
                        nc.sync.dma_start(
                            out=out[w * 128 : w * 128 + rows, :],
                            in_=res[:rows, :],
                        )

            # ---- pipeline ----
            for rnd in range(rounds):
                rows2 = s2["rng"]
                h2vs = [
                    dp.tile(
                        [rows2, 2 * hid], F16, addr_space="Shared",
                        tag=f"h2v{rnd}_{k}", name=f"h2v{rnd}_{k}",
                    )
                    for k in range(AGC)
                ]
                # chunked AllGather: chunk k moves each core's acc1 rows
                # [k*nhalf, (k+1)*nhalf) into its own table tensor.  All but
                # the last chunk are emitted mid-agg1, a GBUFS-lookahead past
                # the last contributing window, so the Pool SEQ reaches them
                # right as their input windows land and the collective runs
                # hidden under agg1's tail.  The last chunk is emitted after
                # a pre-issued batch of next-layer range-0 gathers, which
                # then transfer concurrently with it.
                nhalf = nloc // AGC
                emit_w = {}
                for k in range(AGC - 1):
                    bw = math.ceil((k + 1) * nhalf / 128) - 1 + 4
                    emit_w[min(wn - 1, bw)] = k

                def emit_ag(k):
                    nc.gpsimd.collective_compute(
                        "AllGather",
                        mybir.AluOpType.bypass,
                        replica_groups=[list(range(n_cores))],
                        ins=[acc1loc[k * nhalf : (k + 1) * nhalf, :].opt()],
                        outs=[h2vs[k][:].opt()],
                    )

                def post_w(w):
                    if w in emit_w:
                        emit_ag(emit_w[w])

                agg_pass(s1, [xtab], n_nodes, idx1_t, drel1_t, nrm1_t,
                         nneg1_t, layer=1, post_window=post_w)

                def pre2(ensure):
                    # final AG first so its dispatch isn't gated behind the
                    # prefetch batch; then prefetch range-0 gathers, which
                    # transfer concurrently with the collective.  Leave >=2
                    # buffers for the other queues' first chunks, or window 0
                    # deadlocks waiting on range 1.
                    emit_ag(AGC - 1)
                    t0 = s2["t_start"][0]
                    t1 = s2["t_start"][1]
                    ensure(0, min((GBUFS - 2) * CHUNK, t1 - t0))

                agg_pass(s2, h2vs, npair, idx2_t, drel2_t, nrm2_t, nneg2_t,
                         layer=2, pre_hook=pre2)

    _split_long_waits(nc)
    mybir.codegen_inst_isa_subclasses(nc)
    return nc


def _prepare(x, edge_index, W1, b1, W2, b2, n_nodes=N_NODES, n_cores=N_CORES):
    x = np.asarray(x, dtype=np.float32)
    W1 = np.asarray(W1, dtype=np.float32)
    b1 = np.asarray(b1, dtype=np.float32)
    W2 = np.asarray(W2, dtype=np.float32)
    b2 = np.asarray(b2, dtype=np.float32)

    in_dim = x.shape[1]
    hid = W1.shape[1]
    out_dim = W2.shape[1]

    meta = _preprocess(edge_index, n_nodes, n_cores)

    nc = _build_nc(meta, n_nodes, hid, out_dim, in_dim, n_cores)

    xtab = np.ascontiguousarray(x.astype(np.float16))
    w1h = W1.astype(np.float16)
    w2h = W2.astype(np.float16)
    b1rep = np.tile(b1[None, :], (128, 1)).astype(np.float32)
    b2rep = np.tile(b2[None, :], (128, 1)).astype(np.float32)
    iota = np.tile(np.arange(128, dtype=np.float16)[None, :], (128, 1)).copy()
    iotap = np.arange(128, dtype=np.float32).reshape(128, 1).copy()

    nloc = meta["nloc"]
    wn = meta["wn"]
    npad = wn * 128
    s1, s2 = meta["s1"], meta["s2"]
    in_maps = []
    for c in range(n_cores):
        xs = np.zeros((npad, in_dim), dtype=np.float16)
        xs[:nloc] = xtab[c * nloc : (c + 1) * nloc]
        xloc = np.ascontiguousarray(
            xs.reshape(wn, 128, in_dim).transpose(1, 0, 2).reshape(128, npad)
        )
        in_maps.append(
            {
                "xtab": xtab,
                "xloc": xloc,
                "dinvw": meta["dinvw"][c],
                "iotap": iotap,
                "idx1": s1["idx"][c],
                "drel1": s1["drel"][c],
                "nrm1": s1["nrm"][c],
                "idx2": s2["idx"][c],
                "drel2": s2["drel"][c],
                "nrm2": s2["nrm"][c],
                "w1": w1h,
                "w2": w2h,
                "b1rep": b1rep,
                "b2rep": b2rep,
                "iota": iota,
            }
        )
    return nc, in_maps


def kernel(x, edge_index, W1, b1, W2, b2):
    nc, in_maps = _prepare(x, edge_index, W1, b1, W2, b2)
    res = run_bass_kernel_spmd(nc, in_maps, core_ids=list(range(N_CORES)))
    return np.concatenate([res.results[c]["out"] for c in range(N_CORES)], axis=0)
